# revision 25
# baseline (speedup 1.0000x reference)
"""
Trainium2 Bass kernel for nn_BagModel (segment_reduce, memory-bound).

Model:  h = relu(x @ W1 + b1)          [N, 256]
        feats = h @ W2 + b2            [N, 64]
        pooled = segment_mean(feats)   [B, 64]   (bags = 20 contiguous rows)
        out = pooled @ Wa + ba         [B, 1]

Algebraic restructure used on device (exact up to fp reassociation):
        w2a   = W2 @ Wa                       [256, 1]
        z_i   = relu(x_i @ W1 + b1) @ w2a     scalar per instance
        out_b = (sum_{i in bag b} z_i) / count_b + (b2 @ Wa + ba)

Sharding: pure data-parallel over instances, 8 NeuronCores, 125000
instances = 6250 bags per core (bags never straddle cores for the
reference's inner_ids = i // 20).

Device pipeline (per core, per 4000-instance block):
  - host ships x.T pre-cast to bf16: xT [128=D, 125000]  (halves HBM
    traffic, no on-device transposes)
  - h_T chunks (128 H each): matmul lhsT=W1c, rhs=xT slice -> PSUM f32,
    PSUM->SBUF eviction fused with bias+relu (ScalarE activation /
    VectorE tensor_scalar, alternating for balance) -> bf16 SBUF
  - stage2: z = h_T . w2a via M=1 matmuls column-tiled to PE col-groups
    0/32/64/96, issued in rounds of 4 (concurrent sub-array execution)
  - segment-sum: one VectorE reduce [128, 50, 20] -> [128, 50] per block
    (only psum rows 0/32/64/96 meaningful; garbage lanes never read)
  - final partition-strided DMA gathers rows 0/32/64/96 to DRAM
  - host: divide by per-bag counts (np.bincount), add b2@Wa + ba.
"""

import numpy as np
import ml_dtypes

N = 1_000_000
D = 128
H = 256
F = 64
B = 50_000
NCORES = 8
NS = N // NCORES          # 125000 instances per core
BS = B // NCORES          # 6250 bags per core
BAG = 20                  # instances per bag
BLK = 4000                # instances per block (= 200 bags, strips of 1000)
NFULL = NS // BLK         # 31 full blocks
TAIL = NS - NFULL * BLK   # 1000 instances (= 50 bags)
SUMCOLS = NFULL * 50 + 16  # sums_all cols: 50/block + tail slot

_compiled = {}


def _np_reference(x, inner_ids, W1, b1, W2, b2, Wa, ba):
    """Pure-numpy fallback (used only if inputs don't match the expected
    bag structure).  Replicates jax.ops.segment_sum semantics exactly:
    out-of-range ids are dropped; empty bags give 0/0 = NaN."""
    h = np.maximum(x @ W1 + b1, 0.0)
    feats = (h @ W2 + b2).astype(np.float32)
    ids = inner_ids.astype(np.int64)
    valid = (ids >= 0) & (ids < B)
    sums = np.zeros((B, feats.shape[1]), np.float32)
    np.add.at(sums, ids[valid], feats[valid])
    counts = np.zeros((B, 1), np.float32)
    np.add.at(counts[:, 0], ids[valid], np.float32(1))
    with np.errstate(divide="ignore", invalid="ignore"):
        pooled = sums / counts
    return (pooled @ Wa + ba).astype(np.float32)


def _build_program():
    """Build and compile the 8-core SPMD bass program."""
    import concourse.bacc as bacc
    import concourse.bass as bass
    import concourse.mybir as mybir
    import concourse.tile as tile
    from contextlib import ExitStack

    bf16 = mybir.dt.bfloat16
    f32 = mybir.dt.float32

    nc = bacc.Bacc("TRN2", target_bir_lowering=False, debug=False,
                   num_devices=NCORES)

    xT_d = nc.dram_tensor("xT", (D, NS), bf16, kind="ExternalInput")
    W1_d = nc.dram_tensor("W1b", (D, H), bf16, kind="ExternalInput")
    w2a_d = nc.dram_tensor("w2a", (128, 2), bf16, kind="ExternalInput")
    b1_d = nc.dram_tensor("b1f", (128, 2), f32, kind="ExternalInput")
    out_d = nc.dram_tensor("sums", (BS,), f32, kind="ExternalOutput")

    with tile.TileContext(nc) as tc, ExitStack() as ctx:
        cpool = ctx.enter_context(tc.tile_pool(name="const", bufs=1))
        xpool = ctx.enter_context(tc.tile_pool(name="x", bufs=3))
        hpool = ctx.enter_context(tc.tile_pool(name="h", bufs=3))
        spool = ctx.enter_context(tc.tile_pool(name="s", bufs=1))
        pp = ctx.enter_context(
            tc.tile_pool(name="ps", bufs=2, space=bass.MemorySpace.PSUM))
        zp = ctx.enter_context(
            tc.tile_pool(name="zps", bufs=2, space=bass.MemorySpace.PSUM))

        W1_sb = cpool.tile([D, H], bf16)
        nc.sync.dma_start(W1_sb[:], W1_d.ap())
        w2a_sb = cpool.tile([128, 2], bf16)
        nc.sync.dma_start(w2a_sb[:], w2a_d.ap())
        b1_sb = cpool.tile([128, 2], f32)
        nc.sync.dma_start(b1_sb[:], b1_d.ap())

        sums_all = spool.tile([128, SUMCOLS], f32)
        xT_ap = xT_d.ap()
        Relu = mybir.ActivationFunctionType.Relu
        evict_k = 0  # rotating ACT/DVE assignment counter

        def emit_stage2(hT, blk):
            # stage 2: z accumulated over the two H-chunks; col-group
            # matmuls (group j at PE column 32j), bank-aligned regions.
            # NOTE: a region's start/stop pair must complete before any
            # other start lands in the same PSUM bank (HW-verified), so
            # the chunk loop is innermost.
            zps = zp.tile([128, 1024], f32, tag="z", name="zps")
            if blk < NFULL:
                # group j covers insts [1000j, 1000j+1000), 50 bags.
                # Issue order keeps TWO strips in flight at all times, one
                # open accumulation group per bank (s0->bank0, s1->bank1):
                # strip jA works bank0 while strip jB works bank1, then swap.
                for jA, jB in ((0, 1), (2, 3)):
                    for j, s, c in ((jA, 0, 0), (jB, 1, 0),
                                    (jA, 0, 1), (jB, 1, 1),
                                    (jB, 0, 0), (jA, 1, 0),
                                    (jB, 0, 1), (jA, 1, 1)):
                        nc.tensor.matmul(
                            zps[32 * j:32 * j + 1,
                                512 * s:512 * s + 500],
                            w2a_sb[:, c:c + 1],
                            hT[c][:, 1000 * j + 500 * s:
                                  1000 * j + 500 * s + 500],
                            start=(c == 0), stop=(c == 1),
                            tile_position=(0, 32 * j))
                co = 50 * blk
                nc.vector.reduce_sum(
                    sums_all[:, co:co + 25],
                    zps[:, 0:500].rearrange("p (k t) -> p k t", t=BAG),
                    axis=mybir.AxisListType.X)
                nc.vector.reduce_sum(
                    sums_all[:, co + 25:co + 50],
                    zps[:, 512:1012].rearrange("p (k t) -> p k t", t=BAG),
                    axis=mybir.AxisListType.X)
            else:
                # tail: 1000 insts, strips {260,260,240,240} = 13/13/12/12
                lens = [260, 260, 240, 240]
                offs = [0, 260, 520, 760]
                for j in range(4):
                    for c in range(2):
                        nc.tensor.matmul(
                            zps[32 * j:32 * j + 1, :lens[j]],
                            w2a_sb[:, c:c + 1],
                            hT[c][:, offs[j]:offs[j] + lens[j]],
                            start=(c == 0), stop=(c == 1),
                            tile_position=(0, 32 * j))
                co = 50 * NFULL
                nc.vector.reduce_sum(
                    sums_all[:, co:co + 13],
                    zps[:, :260].rearrange("p (k t) -> p k t", t=BAG),
                    axis=mybir.AxisListType.X)

        # Software pipeline: block b's stage2 is emitted AFTER block b+1's
        # h-matmuls/evictions, so the PE (in-order) never idles waiting for
        # evictions — the next block's h work fills the gap.
        pending = None
        for blk in range(NFULL + 1):
            n = BLK if blk < NFULL else TAIL
            base = blk * BLK

            xt = xpool.tile([D, BLK], bf16, tag="xT")
            nc.sync.dma_start(xt[:, :n], xT_ap[:, base:base + n])

            hT0 = hpool.tile([128, BLK], bf16, tag="hT0")
            hT1 = hpool.tile([128, BLK], bf16, tag="hT1")
            hT = [hT0, hT1]

            # h_T chunks: relu(W1c.T @ xT + b1c); one [128, 1024] PSUM tile
            # (2 banks) per 1024 instances — every matmul-out region must be
            # bank-aligned (HW-verified: bank-crossing outputs corrupt).
            # Single fused eviction per tile.  ACT is ~15% faster per element
            # than DVE and DVE also owns the reduce, so ACT takes 5 of 8.
            for c in range(2):
                for t in range(0, n, 1024):
                    w = min(1024, n - t)
                    ps = pp.tile([128, 1024], f32, tag="hps")
                    for s in range(0, w, 512):
                        sw = min(512, w - s)
                        nc.tensor.matmul(
                            ps[:, s:s + sw],
                            W1_sb[:, 128 * c:128 * (c + 1)],
                            xt[:, t + s:t + s + sw],
                            start=True, stop=True)
                    if evict_k % 8 in (0, 2, 4, 5, 7):
                        nc.scalar.activation(
                            hT[c][:, t:t + w], ps[:, :w], Relu,
                            bias=b1_sb[:, c:c + 1])
                    else:
                        nc.vector.tensor_scalar(
                            out=hT[c][:, t:t + w], in0=ps[:, :w],
                            scalar1=b1_sb[:, c:c + 1], scalar2=0.0,
                            op0=mybir.AluOpType.add, op1=mybir.AluOpType.max)
                    evict_k += 1

            if pending is not None:
                emit_stage2(*pending)
            pending = (hT, blk)
        emit_stage2(*pending)

        # output DMAs.  regular blocks: bag = 200*blk + 50*j + k
        out_ap = out_d.ap()
        nc.sync.dma_start(
            out_ap[0:200 * NFULL].rearrange("(b j k) -> j b k", j=4, k=50),
            sums_all[::32, :50 * NFULL].rearrange("j (b k) -> j b k", k=50))
        # tail: bags 6200 + [0:13) j=0, [13:26) j=1, [26:38) j=2, [38:50) j=3
        co = 50 * NFULL
        nc.sync.dma_start(
            out_ap[200 * NFULL:200 * NFULL + 26].rearrange(
                "(j k) -> j k", j=2),
            sums_all[0:64:32, co:co + 13])
        nc.sync.dma_start(
            out_ap[200 * NFULL + 26:200 * NFULL + 50].rearrange(
                "(j k) -> j k", j=2),
            sums_all[64::32, co:co + 12])

    nc.compile()
    return nc


def _get_program():
    if "nc" not in _compiled:
        _compiled["nc"] = _build_program()
    return _compiled["nc"]


def _make_in_maps(x, W1, b1, W2, Wa):
    w2a = (W2.astype(np.float64) @ Wa.astype(np.float64)).astype(np.float32)
    w2a_in = w2a.reshape(2, 128).T.astype(ml_dtypes.bfloat16).copy()  # [128,2]
    b1_in = b1.reshape(2, 128).T.astype(np.float32).copy()            # [128,2]
    W1_in = W1.astype(ml_dtypes.bfloat16)                             # [128,256]
    xs = x.reshape(NCORES, NS, D)
    in_maps = []
    for c in range(NCORES):
        xT = np.ascontiguousarray(xs[c].T.astype(ml_dtypes.bfloat16))
        in_maps.append({"xT": xT, "W1b": W1_in, "w2a": w2a_in, "b1f": b1_in})
    return in_maps


def _run_device(x, W1, b1, W2, b2, Wa, ba, **spmd_kwargs):
    from concourse import bass_utils

    nc = _get_program()
    in_maps = _make_in_maps(x, W1, b1, W2, Wa)
    res = bass_utils.run_bass_kernel_spmd(
        nc, in_maps, core_ids=list(range(NCORES)), **spmd_kwargs)
    sums = np.concatenate([r["sums"] for r in res.results])           # [B]
    return sums, res


def kernel(x, inner_ids, W1, b1, W2, b2, Wa, ba):
    x = np.asarray(x, np.float32)
    inner_ids = np.asarray(inner_ids)
    W1 = np.asarray(W1, np.float32)
    b1 = np.asarray(b1, np.float32)
    W2 = np.asarray(W2, np.float32)
    b2 = np.asarray(b2, np.float32)
    Wa = np.asarray(Wa, np.float32)
    ba = np.asarray(ba, np.float32)

    expected_ids = np.arange(N, dtype=np.int64) // BAG
    if (x.shape != (N, D) or inner_ids.shape != (N,)
            or not np.array_equal(inner_ids, expected_ids)):
        return _np_reference(x, inner_ids, W1, b1, W2, b2, Wa, ba)

    sums, _ = _run_device(x, W1, b1, W2, b2, Wa, ba)

    counts = np.bincount(inner_ids, minlength=B).astype(np.float32)
    const = (b2.astype(np.float64) @ Wa.astype(np.float64).reshape(-1)
             + ba.astype(np.float64).reshape(-1)[0]).item()
    out = (sums / counts + const).astype(np.float32).reshape(B, 1)
    return out


# revision 26
# speedup vs baseline: 1.2436x; 1.2436x over previous
"""
Trainium2 Bass kernel for nn_BagModel (segment_reduce, memory-bound).

Model:  h = relu(x @ W1 + b1)          [N, 256]
        feats = h @ W2 + b2            [N, 64]
        pooled = segment_mean(feats)   [B, 64]   (bags = 20 contiguous rows)
        out = pooled @ Wa + ba         [B, 1]

Algebraic restructure used on device (exact up to fp reassociation):
        w2a   = W2 @ Wa                       [256, 1]
        z_i   = relu(x_i @ W1 + b1) @ w2a     scalar per instance
        out_b = (sum_{i in bag b} z_i) / count_b + (b2 @ Wa + ba)

Sharding: pure data-parallel over instances, 8 NeuronCores, 125000
instances = 6250 bags per core (bags never straddle cores for the
reference's inner_ids = i // 20).

Device pipeline (per core, per 4000-instance block):
  - host ships x.T pre-cast to bf16: xT [128=D, 125000]  (halves HBM
    traffic, no on-device transposes)
  - h_T chunks (128 H each): matmul lhsT=W1c, rhs=xT slice -> PSUM f32,
    PSUM->SBUF eviction fused with bias+relu (ScalarE activation /
    VectorE tensor_scalar, alternating for balance) -> bf16 SBUF
  - stage2: z = h_T . w2a via M=1 matmuls column-tiled to PE col-groups
    0/32/64/96, issued in rounds of 4 (concurrent sub-array execution)
  - segment-sum: one VectorE reduce [128, 50, 20] -> [128, 50] per block
    (only psum rows 0/32/64/96 meaningful; garbage lanes never read)
  - final partition-strided DMA gathers rows 0/32/64/96 to DRAM
  - host: divide by per-bag counts (np.bincount), add b2@Wa + ba.
"""

import numpy as np
import ml_dtypes

N = 1_000_000
D = 128
H = 256
F = 64
B = 50_000
NCORES = 8
NS = N // NCORES          # 125000 instances per core
BS = B // NCORES          # 6250 bags per core
BAG = 20                  # instances per bag
BLK = 4000                # instances per block (= 200 bags, strips of 1000)
NFULL = NS // BLK         # 31 full blocks
TAIL = NS - NFULL * BLK   # 1000 instances (= 50 bags)
SUMCOLS = NFULL * 50 + 16  # sums_all cols: 50/block + tail slot

_compiled = {}


def _np_reference(x, inner_ids, W1, b1, W2, b2, Wa, ba):
    """Pure-numpy fallback (used only if inputs don't match the expected
    bag structure).  Replicates jax.ops.segment_sum semantics exactly:
    out-of-range ids are dropped; empty bags give 0/0 = NaN."""
    h = np.maximum(x @ W1 + b1, 0.0)
    feats = (h @ W2 + b2).astype(np.float32)
    ids = inner_ids.astype(np.int64)
    valid = (ids >= 0) & (ids < B)
    sums = np.zeros((B, feats.shape[1]), np.float32)
    np.add.at(sums, ids[valid], feats[valid])
    counts = np.zeros((B, 1), np.float32)
    np.add.at(counts[:, 0], ids[valid], np.float32(1))
    with np.errstate(divide="ignore", invalid="ignore"):
        pooled = sums / counts
    return (pooled @ Wa + ba).astype(np.float32)


def _build_program():
    """Build and compile the 8-core SPMD bass program."""
    import concourse.bacc as bacc
    import concourse.bass as bass
    import concourse.mybir as mybir
    import concourse.tile as tile
    from contextlib import ExitStack

    bf16 = mybir.dt.bfloat16
    f32 = mybir.dt.float32

    nc = bacc.Bacc("TRN2", target_bir_lowering=False, debug=False,
                   num_devices=NCORES)

    xT_d = nc.dram_tensor("xT", (D, NS), bf16, kind="ExternalInput")
    W1_d = nc.dram_tensor("W1b", (D, H), bf16, kind="ExternalInput")
    w2a_d = nc.dram_tensor("w2a", (128, 2), bf16, kind="ExternalInput")
    b1_d = nc.dram_tensor("b1f", (128, 2), f32, kind="ExternalInput")
    out_d = nc.dram_tensor("sums", (BS,), f32, kind="ExternalOutput")

    with tile.TileContext(nc) as tc, ExitStack() as ctx:
        cpool = ctx.enter_context(tc.tile_pool(name="const", bufs=1))
        xpool = ctx.enter_context(tc.tile_pool(name="x", bufs=3))
        hpool = ctx.enter_context(tc.tile_pool(name="h", bufs=3))
        spool = ctx.enter_context(tc.tile_pool(name="s", bufs=1))
        pp = ctx.enter_context(
            tc.tile_pool(name="ps", bufs=3, space=bass.MemorySpace.PSUM))
        zp = ctx.enter_context(
            tc.tile_pool(name="zps", bufs=1, space=bass.MemorySpace.PSUM))

        W1_sb = cpool.tile([D, H], bf16)
        nc.sync.dma_start(W1_sb[:], W1_d.ap())
        w2a_sb = cpool.tile([128, 2], bf16)
        nc.sync.dma_start(w2a_sb[:], w2a_d.ap())
        b1_sb = cpool.tile([128, 2], f32)
        nc.sync.dma_start(b1_sb[:], b1_d.ap())

        sums_all = spool.tile([128, SUMCOLS], f32)
        xT_ap = xT_d.ap()
        Relu = mybir.ActivationFunctionType.Relu
        evict_k = 0  # rotating ACT/DVE assignment counter

        def emit_stage2(hT, blk):
            # stage 2: z accumulated over the two H-chunks; col-group
            # matmuls (group j at PE column 32j), bank-aligned regions.
            # NOTE: a region's start/stop pair must complete before any
            # other start lands in the same PSUM bank (HW-verified), so
            # the chunk loop is innermost.
            zps = zp.tile([128, 1024], f32, tag="z", name="zps")
            if blk < NFULL:
                # group j covers insts [1000j, 1000j+1000), 50 bags.
                # Issue order keeps TWO strips in flight at all times, one
                # open accumulation group per bank (s0->bank0, s1->bank1):
                # strip jA works bank0 while strip jB works bank1, then swap.
                for jA, jB in ((0, 1), (2, 3)):
                    for j, s, c in ((jA, 0, 0), (jB, 1, 0),
                                    (jA, 0, 1), (jB, 1, 1),
                                    (jB, 0, 0), (jA, 1, 0),
                                    (jB, 0, 1), (jA, 1, 1)):
                        nc.tensor.matmul(
                            zps[32 * j:32 * j + 1,
                                512 * s:512 * s + 500],
                            w2a_sb[:, c:c + 1],
                            hT[c][:, 1000 * j + 500 * s:
                                  1000 * j + 500 * s + 500],
                            start=(c == 0), stop=(c == 1),
                            tile_position=(0, 32 * j))
                co = 50 * blk
                nc.vector.reduce_sum(
                    sums_all[:, co:co + 25],
                    zps[:, 0:500].rearrange("p (k t) -> p k t", t=BAG),
                    axis=mybir.AxisListType.X)
                nc.vector.reduce_sum(
                    sums_all[:, co + 25:co + 50],
                    zps[:, 512:1012].rearrange("p (k t) -> p k t", t=BAG),
                    axis=mybir.AxisListType.X)
            else:
                # tail: 1000 insts, strips {260,260,240,240} = 13/13/12/12
                lens = [260, 260, 240, 240]
                offs = [0, 260, 520, 760]
                for j in range(4):
                    for c in range(2):
                        nc.tensor.matmul(
                            zps[32 * j:32 * j + 1, :lens[j]],
                            w2a_sb[:, c:c + 1],
                            hT[c][:, offs[j]:offs[j] + lens[j]],
                            start=(c == 0), stop=(c == 1),
                            tile_position=(0, 32 * j))
                co = 50 * NFULL
                nc.vector.reduce_sum(
                    sums_all[:, co:co + 13],
                    zps[:, :260].rearrange("p (k t) -> p k t", t=BAG),
                    axis=mybir.AxisListType.X)

        # Software pipeline: block b's stage2 is emitted AFTER block b+1's
        # h-matmuls/evictions, so the PE (in-order) never idles waiting for
        # evictions — the next block's h work fills the gap.
        pending = None
        for blk in range(NFULL + 1):
            n = BLK if blk < NFULL else TAIL
            base = blk * BLK

            xt = xpool.tile([D, BLK], bf16, tag="xT")
            nc.sync.dma_start(xt[:, :n], xT_ap[:, base:base + n])

            hT0 = hpool.tile([128, BLK], bf16, tag="hT0")
            hT1 = hpool.tile([128, BLK], bf16, tag="hT1")
            hT = [hT0, hT1]

            # h_T chunks: relu(W1c.T @ xT + b1c); one [128, 1024] PSUM tile
            # (2 banks) per 1024 instances — every matmul-out region must be
            # bank-aligned (HW-verified: bank-crossing outputs corrupt).
            # Single fused eviction per tile.  ACT is ~15% faster per element
            # than DVE and DVE also owns the reduce, so ACT takes 5 of 8.
            for c in range(2):
                for t in range(0, n, 1024):
                    w = min(1024, n - t)
                    ps = pp.tile([128, 1024], f32, tag="hps")
                    for s in range(0, w, 512):
                        sw = min(512, w - s)
                        nc.tensor.matmul(
                            ps[:, s:s + sw],
                            W1_sb[:, 128 * c:128 * (c + 1)],
                            xt[:, t + s:t + s + sw],
                            start=True, stop=True)
                    if evict_k % 8 in (0, 2, 4, 5, 7):
                        nc.scalar.activation(
                            hT[c][:, t:t + w], ps[:, :w], Relu,
                            bias=b1_sb[:, c:c + 1])
                    else:
                        nc.vector.tensor_scalar(
                            out=hT[c][:, t:t + w], in0=ps[:, :w],
                            scalar1=b1_sb[:, c:c + 1], scalar2=0.0,
                            op0=mybir.AluOpType.add, op1=mybir.AluOpType.max)
                    evict_k += 1

            if pending is not None:
                emit_stage2(*pending)
            pending = (hT, blk)
        emit_stage2(*pending)

        # output DMAs.  regular blocks: bag = 200*blk + 50*j + k
        out_ap = out_d.ap()
        nc.sync.dma_start(
            out_ap[0:200 * NFULL].rearrange("(b j k) -> j b k", j=4, k=50),
            sums_all[::32, :50 * NFULL].rearrange("j (b k) -> j b k", k=50))
        # tail: bags 6200 + [0:13) j=0, [13:26) j=1, [26:38) j=2, [38:50) j=3
        co = 50 * NFULL
        nc.sync.dma_start(
            out_ap[200 * NFULL:200 * NFULL + 26].rearrange(
                "(j k) -> j k", j=2),
            sums_all[0:64:32, co:co + 13])
        nc.sync.dma_start(
            out_ap[200 * NFULL + 26:200 * NFULL + 50].rearrange(
                "(j k) -> j k", j=2),
            sums_all[64::32, co:co + 12])

    nc.compile()
    return nc


def _get_program():
    if "nc" not in _compiled:
        _compiled["nc"] = _build_program()
    return _compiled["nc"]


def _make_in_maps(x, W1, b1, W2, Wa):
    w2a = (W2.astype(np.float64) @ Wa.astype(np.float64)).astype(np.float32)
    w2a_in = w2a.reshape(2, 128).T.astype(ml_dtypes.bfloat16).copy()  # [128,2]
    b1_in = b1.reshape(2, 128).T.astype(np.float32).copy()            # [128,2]
    W1_in = W1.astype(ml_dtypes.bfloat16)                             # [128,256]
    xs = x.reshape(NCORES, NS, D)
    in_maps = []
    for c in range(NCORES):
        xT = np.ascontiguousarray(xs[c].T.astype(ml_dtypes.bfloat16))
        in_maps.append({"xT": xT, "W1b": W1_in, "w2a": w2a_in, "b1f": b1_in})
    return in_maps


def _run_device(x, W1, b1, W2, b2, Wa, ba, **spmd_kwargs):
    from concourse import bass_utils

    nc = _get_program()
    in_maps = _make_in_maps(x, W1, b1, W2, Wa)
    res = bass_utils.run_bass_kernel_spmd(
        nc, in_maps, core_ids=list(range(NCORES)), **spmd_kwargs)
    sums = np.concatenate([r["sums"] for r in res.results])           # [B]
    return sums, res


def kernel(x, inner_ids, W1, b1, W2, b2, Wa, ba):
    x = np.asarray(x, np.float32)
    inner_ids = np.asarray(inner_ids)
    W1 = np.asarray(W1, np.float32)
    b1 = np.asarray(b1, np.float32)
    W2 = np.asarray(W2, np.float32)
    b2 = np.asarray(b2, np.float32)
    Wa = np.asarray(Wa, np.float32)
    ba = np.asarray(ba, np.float32)

    expected_ids = np.arange(N, dtype=np.int64) // BAG
    if (x.shape != (N, D) or inner_ids.shape != (N,)
            or not np.array_equal(inner_ids, expected_ids)):
        return _np_reference(x, inner_ids, W1, b1, W2, b2, Wa, ba)

    sums, _ = _run_device(x, W1, b1, W2, b2, Wa, ba)

    counts = np.bincount(inner_ids, minlength=B).astype(np.float32)
    const = (b2.astype(np.float64) @ Wa.astype(np.float64).reshape(-1)
             + ba.astype(np.float64).reshape(-1)[0]).item()
    out = (sums / counts + const).astype(np.float32).reshape(B, 1)
    return out


# revision 27
# speedup vs baseline: 1.2455x; 1.0015x over previous
"""
Trainium2 Bass kernel for nn_BagModel (segment_reduce, memory-bound).

Model:  h = relu(x @ W1 + b1)          [N, 256]
        feats = h @ W2 + b2            [N, 64]
        pooled = segment_mean(feats)   [B, 64]   (bags = 20 contiguous rows)
        out = pooled @ Wa + ba         [B, 1]

Algebraic restructure used on device (exact up to fp reassociation):
        w2a   = W2 @ Wa                       [256, 1]
        z_i   = relu(x_i @ W1 + b1) @ w2a     scalar per instance
        out_b = (sum_{i in bag b} z_i) / count_b + (b2 @ Wa + ba)

Sharding: pure data-parallel over instances, 8 NeuronCores, 125000
instances = 6250 bags per core (bags never straddle cores for the
reference's inner_ids = i // 20).

Device pipeline (per core, per 4000-instance block):
  - host ships x.T pre-cast to bf16: xT [128=D, 125000]  (halves HBM
    traffic, no on-device transposes)
  - h_T chunks (128 H each): matmul lhsT=W1c, rhs=xT slice -> PSUM f32,
    PSUM->SBUF eviction fused with bias+relu (ScalarE activation /
    VectorE tensor_scalar, alternating for balance) -> bf16 SBUF
  - stage2: z = h_T . w2a via M=1 matmuls column-tiled to PE col-groups
    0/32/64/96, issued in rounds of 4 (concurrent sub-array execution)
  - segment-sum: one VectorE reduce [128, 50, 20] -> [128, 50] per block
    (only psum rows 0/32/64/96 meaningful; garbage lanes never read)
  - final partition-strided DMA gathers rows 0/32/64/96 to DRAM
  - host: divide by per-bag counts (np.bincount), add b2@Wa + ba.
"""

import numpy as np
import ml_dtypes

N = 1_000_000
D = 128
H = 256
F = 64
B = 50_000
NCORES = 8
NS = N // NCORES          # 125000 instances per core
BS = B // NCORES          # 6250 bags per core
BAG = 20                  # instances per bag
BLK = 4000                # instances per block (= 200 bags, strips of 1000)
NFULL = NS // BLK         # 31 full blocks
TAIL = NS - NFULL * BLK   # 1000 instances (= 50 bags)
SUMCOLS = NFULL * 50 + 16  # sums_all cols: 50/block + tail slot

_compiled = {}


def _np_reference(x, inner_ids, W1, b1, W2, b2, Wa, ba):
    """Pure-numpy fallback (used only if inputs don't match the expected
    bag structure).  Replicates jax.ops.segment_sum semantics exactly:
    out-of-range ids are dropped; empty bags give 0/0 = NaN."""
    h = np.maximum(x @ W1 + b1, 0.0)
    feats = (h @ W2 + b2).astype(np.float32)
    ids = inner_ids.astype(np.int64)
    valid = (ids >= 0) & (ids < B)
    sums = np.zeros((B, feats.shape[1]), np.float32)
    np.add.at(sums, ids[valid], feats[valid])
    counts = np.zeros((B, 1), np.float32)
    np.add.at(counts[:, 0], ids[valid], np.float32(1))
    with np.errstate(divide="ignore", invalid="ignore"):
        pooled = sums / counts
    return (pooled @ Wa + ba).astype(np.float32)


def _build_program():
    """Build and compile the 8-core SPMD bass program."""
    import concourse.bacc as bacc
    import concourse.bass as bass
    import concourse.mybir as mybir
    import concourse.tile as tile
    from contextlib import ExitStack

    bf16 = mybir.dt.float16  # fp16: same PE/DVE speed class as bf16, 3 more mantissa bits
    f32 = mybir.dt.float32

    nc = bacc.Bacc("TRN2", target_bir_lowering=False, debug=False,
                   num_devices=NCORES)

    xT_d = nc.dram_tensor("xT", (D, NS), bf16, kind="ExternalInput")
    W1_d = nc.dram_tensor("W1b", (D, H), bf16, kind="ExternalInput")
    w2a_d = nc.dram_tensor("w2a", (128, 2), bf16, kind="ExternalInput")
    b1_d = nc.dram_tensor("b1f", (128, 2), f32, kind="ExternalInput")
    out_d = nc.dram_tensor("sums", (BS,), f32, kind="ExternalOutput")

    with tile.TileContext(nc) as tc, ExitStack() as ctx:
        cpool = ctx.enter_context(tc.tile_pool(name="const", bufs=1))
        xpool = ctx.enter_context(tc.tile_pool(name="x", bufs=3))
        hpool = ctx.enter_context(tc.tile_pool(name="h", bufs=3))
        spool = ctx.enter_context(tc.tile_pool(name="s", bufs=1))
        pp = ctx.enter_context(
            tc.tile_pool(name="ps", bufs=3, space=bass.MemorySpace.PSUM))
        zp = ctx.enter_context(
            tc.tile_pool(name="zps", bufs=1, space=bass.MemorySpace.PSUM))

        W1_sb = cpool.tile([D, H], bf16)
        nc.sync.dma_start(W1_sb[:], W1_d.ap())
        w2a_sb = cpool.tile([128, 2], bf16)
        nc.sync.dma_start(w2a_sb[:], w2a_d.ap())
        b1_sb = cpool.tile([128, 2], f32)
        nc.sync.dma_start(b1_sb[:], b1_d.ap())

        sums_all = spool.tile([128, SUMCOLS], f32)
        xT_ap = xT_d.ap()
        Relu = mybir.ActivationFunctionType.Relu
        evict_k = 0  # rotating ACT/DVE assignment counter

        def emit_stage2(hT, blk):
            # stage 2: z accumulated over the two H-chunks; col-group
            # matmuls (group j at PE column 32j), bank-aligned regions.
            # NOTE: a region's start/stop pair must complete before any
            # other start lands in the same PSUM bank (HW-verified), so
            # the chunk loop is innermost.
            zps = zp.tile([128, 1024], f32, tag="z", name="zps")
            if blk < NFULL:
                # group j covers insts [1000j, 1000j+1000), 50 bags.
                # Issue order keeps TWO strips in flight at all times, one
                # open accumulation group per bank (s0->bank0, s1->bank1):
                # strip jA works bank0 while strip jB works bank1, then swap.
                for jA, jB in ((0, 1), (2, 3)):
                    for j, s, c in ((jA, 0, 0), (jB, 1, 0),
                                    (jA, 0, 1), (jB, 1, 1),
                                    (jB, 0, 0), (jA, 1, 0),
                                    (jB, 0, 1), (jA, 1, 1)):
                        nc.tensor.matmul(
                            zps[32 * j:32 * j + 1,
                                512 * s:512 * s + 500],
                            w2a_sb[:, c:c + 1],
                            hT[c][:, 1000 * j + 500 * s:
                                  1000 * j + 500 * s + 500],
                            start=(c == 0), stop=(c == 1),
                            tile_position=(0, 32 * j))
                co = 50 * blk
                nc.vector.reduce_sum(
                    sums_all[:, co:co + 25],
                    zps[:, 0:500].rearrange("p (k t) -> p k t", t=BAG),
                    axis=mybir.AxisListType.X)
                nc.vector.reduce_sum(
                    sums_all[:, co + 25:co + 50],
                    zps[:, 512:1012].rearrange("p (k t) -> p k t", t=BAG),
                    axis=mybir.AxisListType.X)
            else:
                # tail: 1000 insts, strips {260,260,240,240} = 13/13/12/12
                lens = [260, 260, 240, 240]
                offs = [0, 260, 520, 760]
                for j in range(4):
                    for c in range(2):
                        nc.tensor.matmul(
                            zps[32 * j:32 * j + 1, :lens[j]],
                            w2a_sb[:, c:c + 1],
                            hT[c][:, offs[j]:offs[j] + lens[j]],
                            start=(c == 0), stop=(c == 1),
                            tile_position=(0, 32 * j))
                co = 50 * NFULL
                nc.vector.reduce_sum(
                    sums_all[:, co:co + 13],
                    zps[:, :260].rearrange("p (k t) -> p k t", t=BAG),
                    axis=mybir.AxisListType.X)

        # Software pipeline: block b's stage2 is emitted AFTER block b+1's
        # h-matmuls/evictions, so the PE (in-order) never idles waiting for
        # evictions — the next block's h work fills the gap.
        pending = None
        for blk in range(NFULL + 1):
            n = BLK if blk < NFULL else TAIL
            base = blk * BLK

            xt = xpool.tile([D, BLK], bf16, tag="xT")
            nc.sync.dma_start(xt[:, :n], xT_ap[:, base:base + n])

            hT0 = hpool.tile([128, BLK], bf16, tag="hT0")
            hT1 = hpool.tile([128, BLK], bf16, tag="hT1")
            hT = [hT0, hT1]

            # h_T chunks: relu(W1c.T @ xT + b1c); one [128, 1024] PSUM tile
            # (2 banks) per 1024 instances — every matmul-out region must be
            # bank-aligned (HW-verified: bank-crossing outputs corrupt).
            # Single fused eviction per tile.  ACT is ~15% faster per element
            # than DVE and DVE also owns the reduce, so ACT takes 5 of 8.
            for c in range(2):
                for t in range(0, n, 1024):
                    w = min(1024, n - t)
                    ps = pp.tile([128, 1024], f32, tag="hps")
                    for s in range(0, w, 512):
                        sw = min(512, w - s)
                        nc.tensor.matmul(
                            ps[:, s:s + sw],
                            W1_sb[:, 128 * c:128 * (c + 1)],
                            xt[:, t + s:t + s + sw],
                            start=True, stop=True)
                    if evict_k % 8 in (0, 2, 4, 5, 7):
                        nc.scalar.activation(
                            hT[c][:, t:t + w], ps[:, :w], Relu,
                            bias=b1_sb[:, c:c + 1])
                    else:
                        nc.vector.tensor_scalar(
                            out=hT[c][:, t:t + w], in0=ps[:, :w],
                            scalar1=b1_sb[:, c:c + 1], scalar2=0.0,
                            op0=mybir.AluOpType.add, op1=mybir.AluOpType.max)
                    evict_k += 1

            if pending is not None:
                emit_stage2(*pending)
            pending = (hT, blk)
        emit_stage2(*pending)

        # output DMAs.  regular blocks: bag = 200*blk + 50*j + k
        out_ap = out_d.ap()
        nc.sync.dma_start(
            out_ap[0:200 * NFULL].rearrange("(b j k) -> j b k", j=4, k=50),
            sums_all[::32, :50 * NFULL].rearrange("j (b k) -> j b k", k=50))
        # tail: bags 6200 + [0:13) j=0, [13:26) j=1, [26:38) j=2, [38:50) j=3
        co = 50 * NFULL
        nc.sync.dma_start(
            out_ap[200 * NFULL:200 * NFULL + 26].rearrange(
                "(j k) -> j k", j=2),
            sums_all[0:64:32, co:co + 13])
        nc.sync.dma_start(
            out_ap[200 * NFULL + 26:200 * NFULL + 50].rearrange(
                "(j k) -> j k", j=2),
            sums_all[64::32, co:co + 12])

    nc.compile()
    return nc


def _get_program():
    if "nc" not in _compiled:
        _compiled["nc"] = _build_program()
    return _compiled["nc"]


def _make_in_maps(x, W1, b1, W2, Wa):
    w2a = (W2.astype(np.float64) @ Wa.astype(np.float64)).astype(np.float32)
    w2a_in = w2a.reshape(2, 128).T.astype(np.float16).copy()  # [128,2]
    b1_in = b1.reshape(2, 128).T.astype(np.float32).copy()            # [128,2]
    W1_in = W1.astype(np.float16)                             # [128,256]
    xs = x.reshape(NCORES, NS, D)
    in_maps = []
    for c in range(NCORES):
        xT = np.ascontiguousarray(xs[c].T.astype(np.float16))
        in_maps.append({"xT": xT, "W1b": W1_in, "w2a": w2a_in, "b1f": b1_in})
    return in_maps


def _run_device(x, W1, b1, W2, b2, Wa, ba, **spmd_kwargs):
    from concourse import bass_utils

    nc = _get_program()
    in_maps = _make_in_maps(x, W1, b1, W2, Wa)
    res = bass_utils.run_bass_kernel_spmd(
        nc, in_maps, core_ids=list(range(NCORES)), **spmd_kwargs)
    sums = np.concatenate([r["sums"] for r in res.results])           # [B]
    return sums, res


def kernel(x, inner_ids, W1, b1, W2, b2, Wa, ba):
    x = np.asarray(x, np.float32)
    inner_ids = np.asarray(inner_ids)
    W1 = np.asarray(W1, np.float32)
    b1 = np.asarray(b1, np.float32)
    W2 = np.asarray(W2, np.float32)
    b2 = np.asarray(b2, np.float32)
    Wa = np.asarray(Wa, np.float32)
    ba = np.asarray(ba, np.float32)

    expected_ids = np.arange(N, dtype=np.int64) // BAG
    if (x.shape != (N, D) or inner_ids.shape != (N,)
            or not np.array_equal(inner_ids, expected_ids)):
        return _np_reference(x, inner_ids, W1, b1, W2, b2, Wa, ba)

    sums, _ = _run_device(x, W1, b1, W2, b2, Wa, ba)

    counts = np.bincount(inner_ids, minlength=B).astype(np.float32)
    const = (b2.astype(np.float64) @ Wa.astype(np.float64).reshape(-1)
             + ba.astype(np.float64).reshape(-1)[0]).item()
    out = (sums / counts + const).astype(np.float32).reshape(B, 1)
    return out


# revision 29
# speedup vs baseline: 1.2537x; 1.0066x over previous
"""
Trainium2 Bass kernel for nn_BagModel (segment_reduce, memory-bound).

Model:  h = relu(x @ W1 + b1)          [N, 256]
        feats = h @ W2 + b2            [N, 64]
        pooled = segment_mean(feats)   [B, 64]   (bags = 20 contiguous rows)
        out = pooled @ Wa + ba         [B, 1]

Algebraic restructure used on device (exact up to fp reassociation):
        w2a   = W2 @ Wa                       [256, 1]
        z_i   = relu(x_i @ W1 + b1) @ w2a     scalar per instance
        out_b = (sum_{i in bag b} z_i) / count_b + (b2 @ Wa + ba)

Sharding: pure data-parallel over instances, 8 NeuronCores, 125000
instances = 6250 bags per core (bags never straddle cores for the
reference's inner_ids = i // 20).

Device pipeline (per core, per 4000-instance block):
  - host ships x.T pre-cast to fp16: xT [128=D, 125000]  (halves HBM
    traffic, no on-device transposes)
  - h_T chunks (128 H each): matmul lhsT=W1c, rhs=xT slice -> PSUM f32,
    PSUM->SBUF eviction fused with bias+relu (ScalarE activation /
    VectorE tensor_scalar, alternating for balance) -> fp16 SBUF
  - stage2: z = h_T . w2a via M=1 matmuls column-tiled to PE col-groups
    0/32/64/96, issued in rounds of 4 (concurrent sub-array execution)
  - segment-sum: one VectorE reduce [128, 50, 20] -> [128, 50] per block
    (only psum rows 0/32/64/96 meaningful; garbage lanes never read)
  - final partition-strided DMA gathers rows 0/32/64/96 to DRAM
  - host: divide by per-bag counts (np.bincount), add b2@Wa + ba.
"""

import numpy as np
import ml_dtypes

N = 1_000_000
D = 128
H = 256
F = 64
B = 50_000
NCORES = 8
NS = N // NCORES          # 125000 instances per core
BS = B // NCORES          # 6250 bags per core
BAG = 20                  # instances per bag
BLK = 4000                # instances per block (= 200 bags, strips of 1000)
NFULL = NS // BLK         # 31 full blocks
TAIL = NS - NFULL * BLK   # 1000 instances (= 50 bags)
SUMCOLS = NFULL * 50 + 16  # sums_all cols: 50/block + tail slot

_compiled = {}


def _np_reference(x, inner_ids, W1, b1, W2, b2, Wa, ba):
    """Pure-numpy fallback (used only if inputs don't match the expected
    bag structure).  Replicates jax.ops.segment_sum semantics exactly:
    out-of-range ids are dropped; empty bags give 0/0 = NaN."""
    h = np.maximum(x @ W1 + b1, 0.0)
    feats = (h @ W2 + b2).astype(np.float32)
    ids = inner_ids.astype(np.int64)
    valid = (ids >= 0) & (ids < B)
    sums = np.zeros((B, feats.shape[1]), np.float32)
    np.add.at(sums, ids[valid], feats[valid])
    counts = np.zeros((B, 1), np.float32)
    np.add.at(counts[:, 0], ids[valid], np.float32(1))
    with np.errstate(divide="ignore", invalid="ignore"):
        pooled = sums / counts
    return (pooled @ Wa + ba).astype(np.float32)


def _build_program():
    """Build and compile the 8-core SPMD bass program."""
    import concourse.bacc as bacc
    import concourse.bass as bass
    import concourse.mybir as mybir
    import concourse.tile as tile
    from contextlib import ExitStack

    bf16 = mybir.dt.float16  # fp16: same PE/DVE speed class as bf16, 3 more mantissa bits
    f32 = mybir.dt.float32

    nc = bacc.Bacc("TRN2", target_bir_lowering=False, debug=False,
                   num_devices=NCORES)

    xT_d = nc.dram_tensor("xT", (D, NS), bf16, kind="ExternalInput")
    W1_d = nc.dram_tensor("W1b", (D, H), bf16, kind="ExternalInput")
    w2a_d = nc.dram_tensor("w2a", (128, 2), bf16, kind="ExternalInput")
    b1_d = nc.dram_tensor("b1f", (128, 2), f32, kind="ExternalInput")
    out_d = nc.dram_tensor("sums", (BS,), f32, kind="ExternalOutput")

    with tile.TileContext(nc) as tc, ExitStack() as ctx:
        cpool = ctx.enter_context(tc.tile_pool(name="const", bufs=1))
        xpool = ctx.enter_context(tc.tile_pool(name="x", bufs=3))
        hpool = ctx.enter_context(tc.tile_pool(name="h", bufs=3))
        spool = ctx.enter_context(tc.tile_pool(name="s", bufs=1))
        pp = ctx.enter_context(
            tc.tile_pool(name="ps", bufs=3, space=bass.MemorySpace.PSUM))
        zp = ctx.enter_context(
            tc.tile_pool(name="zps", bufs=1, space=bass.MemorySpace.PSUM))

        # first x-block DMA leads the HWDGE FIFO so the PE's first matmul
        # isn't delayed behind the (non-urgent) weight transfers
        xt0 = xpool.tile([D, BLK], bf16, tag="xT", name="xt0")
        nc.sync.dma_start(xt0[:, :BLK], xT_d.ap()[:, 0:BLK])
        W1_sb = cpool.tile([D, H], bf16)
        nc.sync.dma_start(W1_sb[:], W1_d.ap())
        w2a_sb = cpool.tile([128, 2], bf16)
        nc.sync.dma_start(w2a_sb[:], w2a_d.ap())
        b1_sb = cpool.tile([128, 2], f32)
        nc.sync.dma_start(b1_sb[:], b1_d.ap())

        sums_all = spool.tile([128, SUMCOLS], f32)
        xT_ap = xT_d.ap()
        Relu = mybir.ActivationFunctionType.Relu
        evict_k = 0  # rotating ACT/DVE assignment counter

        def emit_stage2(hT, blk):
            # stage 2: z accumulated over the two H-chunks; col-group
            # matmuls (group j at PE column 32j), bank-aligned regions.
            # NOTE: a region's start/stop pair must complete before any
            # other start lands in the same PSUM bank (HW-verified), so
            # the chunk loop is innermost.
            zps = zp.tile([128, 1024], f32, tag="z", name="zps")
            if blk < NFULL:
                # group j covers insts [1000j, 1000j+1000), 50 bags.
                # Issue order keeps TWO strips in flight at all times, one
                # open accumulation group per bank (s0->bank0, s1->bank1):
                # strip jA works bank0 while strip jB works bank1, then swap.
                for jA, jB in ((0, 1), (2, 3)):
                    for j, s, c in ((jA, 0, 0), (jB, 1, 0),
                                    (jA, 0, 1), (jB, 1, 1),
                                    (jB, 0, 0), (jA, 1, 0),
                                    (jB, 0, 1), (jA, 1, 1)):
                        nc.tensor.matmul(
                            zps[32 * j:32 * j + 1,
                                512 * s:512 * s + 500],
                            w2a_sb[:, c:c + 1],
                            hT[c][:, 1000 * j + 500 * s:
                                  1000 * j + 500 * s + 500],
                            start=(c == 0), stop=(c == 1),
                            tile_position=(0, 32 * j))
                co = 50 * blk
                nc.vector.reduce_sum(
                    sums_all[:, co:co + 25],
                    zps[:, 0:500].rearrange("p (k t) -> p k t", t=BAG),
                    axis=mybir.AxisListType.X)
                nc.vector.reduce_sum(
                    sums_all[:, co + 25:co + 50],
                    zps[:, 512:1012].rearrange("p (k t) -> p k t", t=BAG),
                    axis=mybir.AxisListType.X)
            else:
                # tail: 1000 insts, strips {260,260,240,240} = 13/13/12/12
                lens = [260, 260, 240, 240]
                offs = [0, 260, 520, 760]
                for j in range(4):
                    for c in range(2):
                        nc.tensor.matmul(
                            zps[32 * j:32 * j + 1, :lens[j]],
                            w2a_sb[:, c:c + 1],
                            hT[c][:, offs[j]:offs[j] + lens[j]],
                            start=(c == 0), stop=(c == 1),
                            tile_position=(0, 32 * j))
                co = 50 * NFULL
                nc.vector.reduce_sum(
                    sums_all[:, co:co + 13],
                    zps[:, :260].rearrange("p (k t) -> p k t", t=BAG),
                    axis=mybir.AxisListType.X)

        # Software pipeline: block b's stage2 is emitted AFTER block b+1's
        # h-matmuls/evictions, so the PE (in-order) never idles waiting for
        # evictions — the next block's h work fills the gap.
        pending = None
        for blk in range(NFULL + 1):
            n = BLK if blk < NFULL else TAIL
            base = blk * BLK

            if blk == 0:
                xt = xt0
            else:
                xt = xpool.tile([D, BLK], bf16, tag="xT")
                nc.sync.dma_start(xt[:, :n], xT_ap[:, base:base + n])

            hT0 = hpool.tile([128, BLK], bf16, tag="hT0")
            hT1 = hpool.tile([128, BLK], bf16, tag="hT1")
            hT = [hT0, hT1]

            # h_T chunks: relu(W1c.T @ xT + b1c); one [128, 1024] PSUM tile
            # (2 banks) per 1024 instances — every matmul-out region must be
            # bank-aligned (HW-verified: bank-crossing outputs corrupt).
            # Single fused eviction per tile.  ACT is ~15% faster per element
            # than DVE and DVE also owns the reduce, so ACT takes 5 of 8.
            for c in range(2):
                for t in range(0, n, 1024):
                    w = min(1024, n - t)
                    ps = pp.tile([128, 1024], f32, tag="hps")
                    for s in range(0, w, 512):
                        sw = min(512, w - s)
                        nc.tensor.matmul(
                            ps[:, s:s + sw],
                            W1_sb[:, 128 * c:128 * (c + 1)],
                            xt[:, t + s:t + s + sw],
                            start=True, stop=True)
                    if evict_k % 8 in (0, 2, 4, 5, 7):
                        nc.scalar.activation(
                            hT[c][:, t:t + w], ps[:, :w], Relu,
                            bias=b1_sb[:, c:c + 1])
                    else:
                        nc.vector.tensor_scalar(
                            out=hT[c][:, t:t + w], in0=ps[:, :w],
                            scalar1=b1_sb[:, c:c + 1], scalar2=0.0,
                            op0=mybir.AluOpType.add, op1=mybir.AluOpType.max)
                    evict_k += 1

            if pending is not None:
                emit_stage2(*pending)
            pending = (hT, blk)
        emit_stage2(*pending)

        # output DMAs.  regular blocks: bag = 200*blk + 50*j + k
        out_ap = out_d.ap()
        nc.sync.dma_start(
            out_ap[0:200 * NFULL].rearrange("(b j k) -> j b k", j=4, k=50),
            sums_all[::32, :50 * NFULL].rearrange("j (b k) -> j b k", k=50))
        # tail: bags 6200 + [0:13) j=0, [13:26) j=1, [26:38) j=2, [38:50) j=3
        co = 50 * NFULL
        nc.sync.dma_start(
            out_ap[200 * NFULL:200 * NFULL + 26].rearrange(
                "(j k) -> j k", j=2),
            sums_all[0:64:32, co:co + 13])
        nc.sync.dma_start(
            out_ap[200 * NFULL + 26:200 * NFULL + 50].rearrange(
                "(j k) -> j k", j=2),
            sums_all[64::32, co:co + 12])

    nc.compile()
    return nc


def _get_program():
    if "nc" not in _compiled:
        _compiled["nc"] = _build_program()
    return _compiled["nc"]


def _make_in_maps(x, W1, b1, W2, Wa):
    w2a = (W2.astype(np.float64) @ Wa.astype(np.float64)).astype(np.float32)
    w2a_in = w2a.reshape(2, 128).T.astype(np.float16).copy()  # [128,2]
    b1_in = b1.reshape(2, 128).T.astype(np.float32).copy()            # [128,2]
    W1_in = W1.astype(np.float16)                             # [128,256]
    xs = x.reshape(NCORES, NS, D)
    in_maps = []
    for c in range(NCORES):
        xT = np.ascontiguousarray(xs[c].T.astype(np.float16))
        in_maps.append({"xT": xT, "W1b": W1_in, "w2a": w2a_in, "b1f": b1_in})
    return in_maps


def _run_device(x, W1, b1, W2, b2, Wa, ba, **spmd_kwargs):
    from concourse import bass_utils

    nc = _get_program()
    in_maps = _make_in_maps(x, W1, b1, W2, Wa)
    res = bass_utils.run_bass_kernel_spmd(
        nc, in_maps, core_ids=list(range(NCORES)), **spmd_kwargs)
    sums = np.concatenate([r["sums"] for r in res.results])           # [B]
    return sums, res


def kernel(x, inner_ids, W1, b1, W2, b2, Wa, ba):
    x = np.asarray(x, np.float32)
    inner_ids = np.asarray(inner_ids)
    W1 = np.asarray(W1, np.float32)
    b1 = np.asarray(b1, np.float32)
    W2 = np.asarray(W2, np.float32)
    b2 = np.asarray(b2, np.float32)
    Wa = np.asarray(Wa, np.float32)
    ba = np.asarray(ba, np.float32)

    expected_ids = np.arange(N, dtype=np.int64) // BAG
    if (x.shape != (N, D) or inner_ids.shape != (N,)
            or not np.array_equal(inner_ids, expected_ids)):
        return _np_reference(x, inner_ids, W1, b1, W2, b2, Wa, ba)

    sums, _ = _run_device(x, W1, b1, W2, b2, Wa, ba)

    counts = np.bincount(inner_ids, minlength=B).astype(np.float32)
    const = (b2.astype(np.float64) @ Wa.astype(np.float64).reshape(-1)
             + ba.astype(np.float64).reshape(-1)[0]).item()
    out = (sums / counts + const).astype(np.float32).reshape(B, 1)
    return out
